# revision 17
# baseline (speedup 1.0000x reference)
"""Trainium2 Bass kernel for BSplineBasis (degree-3, 64 uniform-ish knots).

Math (same truncated-power form as the previous version): for normalized
y = xn - 0.5,

    out[n, i] = A_i(y) + sum_m J[m,i] * relu(y - kappa'_m)^3

evaluated per 490-point tile, two tiles paired via block-diagonal weights
(M = 120 <= 128):
  MM1 (K=6, fp32r): cubic relu-args + y-power pass-throughs
  clamp:            stack = max(G + bias, minclamp)   [DVE or Pool]
  MM2 (K=120, fp32r): out.T = blockdiag(W2, W2).T @ stack -> PSUM
  evict:            PSUM -> SBUF staging in bf16      [ACT or DVE]
  out DMA:          bf16 staging -> DRAM (halves output bytes vs f32)

Perf structure vs the previous version:
  - output shipped bf16 (l2 tolerance has ~20x margin), halving the
    dominant DMA term
  - the two full-volume elementwise passes (clamp, evict) are split
    across DVE / Pool / ACT so no single engine exceeds the PE's ~26us
  - global min/max: full x loaded in 8 chunks; DVE min-reduces each chunk,
    Pool folds a running max; shard block is the first 490 columns of the
    same buffer (no separate shard load)
  - dummy matmuls during the min/max phase keep the PE clock ramped so
    real matmuls run at full rate
"""
import os
import sys

import numpy as np

if "/opt/trn_rl_repo" not in sys.path:
    sys.path.insert(0, "/opt/trn_rl_repo")

DEGREE = 3
NUM_KNOTS = 64
NB = NUM_KNOTS - DEGREE - 1          # 60 basis elements
N_POINTS = 500_000
N_CORES = 8
SHARD = N_POINTS // N_CORES          # 62500
TILE_W = 490                          # points per matmul tile (even: fp32r)
N_TILES = 128                         # 128 * 490 = 62720 >= SHARD
SHARD_PAD = N_TILES * TILE_W          # 62720
NF = 56                               # truncated-power features
FULL_COLS = 3908                      # 128 * 3908 = 500224 >= N_POINTS
FULL_PAD = 128 * FULL_COLS


# ----------------------------------------------------------------- host math
def _piece_poly_coeffs(knots, i, ell):
    """Monomial coeffs (len 4) of the de Boor piece for element i, interval
    ell in [3,6] — replicates the reference recursion, fit exactly in f64."""
    k = DEGREE
    seg = knots[i:i + k + 2]
    T = np.concatenate([np.full(k, seg[0] - 1.0), seg, np.full(k, seg[-1] + 1.0)])

    def eval_at(x):
        res = [np.float64(1.0)] + [np.float64(0.0)] * k
        for j in range(1, k + 1):
            hh = list(res[:j])
            res[0] = np.float64(0.0)
            for n in range(1, j + 1):
                tb, ta = T[ell + n], T[ell + n - j]
                den = tb - ta
                w = 0.0 if den == 0 else hh[n - 1] / den
                res[n - 1] = res[n - 1] + w * (tb - x)
                res[n] = w * (x - ta)
        return res[2 * k - ell]

    xs = np.linspace(-0.3, 1.3, 5)
    V = np.vander(xs, 4, increasing=True)
    return np.linalg.lstsq(V, np.array([eval_at(x) for x in xs]), rcond=None)[0]


def build_tables(knots):
    """A [4,60], CUBE4 [4,56], J [56,60] for the truncated-power form."""
    knots = np.asarray(knots, np.float64)
    P = [[_piece_poly_coeffs(knots, i, p + 3) for p in range(4)] for i in range(NB)]

    def p_of(s, i):
        return int(np.clip(s - i - 1, 0, 3))

    A = np.zeros((4, NB))
    for i in range(NB):
        A[:, i] = P[i][p_of(4, i)]

    ms = list(range(4, 60))
    J = np.zeros((len(ms), NB))
    for f, m in enumerate(ms):
        for i in range(NB):
            pb, pa = p_of(m, i), p_of(m + 1, i)
            if pa != pb:
                J[f, i] = (P[i][pa] - P[i][pb])[3]

    kaps = knots[4:60]
    CUBE4 = np.stack([-kaps**3, 3 * kaps**2, -3 * kaps, np.ones(NF)], 0)
    # reference row at xn == 1.0 exactly (searchsorted s = 64 there)
    row1 = np.array([np.polyval(P[i][3][::-1], 1.0) for i in range(NB)])
    return A, CUBE4, J, row1


CENTER = 0.5  # powers are of y = xn - CENTER to reduce monomial cancellation


def _shift_poly(c, h):
    """coeffs of p(y + h) given coeffs c of p(x), low->high, exact in f64."""
    from math import comb
    out = np.zeros_like(c)
    for q in range(4):
        for r in range(q + 1):
            out[r] += c[q] * comb(q, r) * h ** (q - r)
    return out


def _make_const_arrays(knots):
    A, CUBE4, J, row1 = build_tables(knots)
    # re-express in y = xn - CENTER
    A = np.stack([_shift_poly(A[:, i], CENTER) for i in range(NB)], 1)
    kaps = np.asarray(knots, np.float64)[4:60] - CENTER
    CUBE4 = np.stack([-kaps**3, 3 * kaps**2, -3 * kaps, np.ones(NF)], 0)
    # MM1 (K=6: powers of tile a, powers of tile b): cols 0-55 produce the
    # non-constant part of (y-kappa)^3, cols 56-59 pass powers through
    # (constant terms come in via the clamp bias).
    c3 = np.zeros((3, NB), np.float32)
    c3[:, :NF] = CUBE4[1:4, :]
    for q in range(1, 4):
        c3[q - 1, NF + q] = 1.0
    cube3x = np.zeros((6, 2 * NB), np.float32)
    cube3x[0:3, :NB] = c3
    cube3x[3:6, NB:] = c3
    # The clamp is a single Relu(in + bias) on ALL 120 rows (so either DVE
    # or ACT can run it): cube rows get -kappa'^3 and genuinely relu; the
    # y^q pass-through rows get +0.5 (y^q >= -0.5 always, exact in fp32, so
    # relu is the identity there); the ones row gets 1.0. The A-contraction
    # then sees (y^q + 0.5), and the spurious 0.5*(A1+A2+A3) constant is
    # subtracted via the evict bias evb.
    bias = np.zeros((2 * NB, 1), np.float32)
    for h in (0, NB):
        bias[h:h + NF, 0] = CUBE4[0, :]  # -kappa'^3
        bias[h + NF, 0] = 1.0            # the y^0 == 1 row
        bias[h + NF + 1:h + NB, 0] = 0.5
    # MM2 weights: rows 0-55 = J band, rows 56-59 = base cubic A, blockdiag
    w2s = np.zeros((NB, NB), np.float32)
    w2s[:NF, :] = J
    w2s[NF:, :] = A
    w2 = np.zeros((2 * NB, 2 * NB), np.float32)
    w2[:NB, :NB] = w2s
    w2[NB:, NB:] = w2s
    evb = np.zeros((2 * NB, 1), np.float32)
    corr = -0.5 * (A[1, :] + A[2, :] + A[3, :])
    evb[:NB, 0] = corr
    evb[NB:, 0] = corr
    return cube3x, bias, evb, w2, row1


# -------------------------------------------------------------- bass program
_CACHE = {}

GROUP = 8          # 2-pair iterations per output staging buffer
N_CHUNKS = 8       # full-x DMA/reduce chunks
CHW = 488          # chunk width (8*488 = 3904; 4-col tail folded separately)


def _build_nc():
    import concourse.tile as tile
    from concourse import bacc, mybir

    f32 = mybir.dt.float32
    f32r = mybir.dt.float32r
    bf16 = mybir.dt.bfloat16

    nc = bacc.Bacc("TRN2", target_bir_lowering=False, debug=False)
    x_full = nc.declare_dram_parameter("x_full", [128, FULL_COLS], f32, isOutput=False)
    cube3x_d = nc.declare_dram_parameter("cube3x", [6, 2 * NB], f32, isOutput=False)
    bias_d = nc.declare_dram_parameter("bias124", [2 * NB, 1], f32, isOutput=False)
    evb_d = nc.declare_dram_parameter("evb", [2 * NB, 1], f32, isOutput=False)
    w2_d = nc.declare_dram_parameter("w2", [2 * NB, 2 * NB], f32, isOutput=False)
    out_t = nc.declare_dram_parameter("out_t", [NB, SHARD_PAD], bf16, isOutput=True)

    W = TILE_W
    BANK = 512

    with tile.TileContext(nc) as tc:
        with (
            tc.tile_pool(name="big", bufs=1) as big_pool,
            tc.tile_pool(name="consts", bufs=1) as const_pool,
            tc.tile_pool(name="xrows", bufs=6) as xrows_pool,
            tc.tile_pool(name="stack", bufs=4) as stack_pool,
            tc.tile_pool(name="stage", bufs=3) as stage_pool,
            tc.tile_pool(name="gpsum", bufs=2, space="PSUM") as gpsum_pool,
            tc.tile_pool(name="opsum", bufs=2, space="PSUM") as opsum_pool,
            tc.tile_pool(name="tiny", bufs=1) as tiny_pool,
        ):
            # ---- constants into SBUF
            cu = const_pool.tile([38, 2 * NB], f32r)
            nc.scalar.dma_start(cu[0:6, :], cube3x_d[:, :].bitcast(f32r))
            nc.scalar.dma_start(cu[32:38, :], cube3x_d[:, :].bitcast(f32r))
            bias = const_pool.tile([2 * NB, 1], f32)
            nc.scalar.dma_start(bias[:, :], bias_d[:, :])
            evb = const_pool.tile([2 * NB, 1], f32)
            nc.scalar.dma_start(evb[:, :], evb_d[:, :])
            w2t = const_pool.tile([2 * NB, 2 * NB], f32r)
            nc.scalar.dma_start(w2t[:, :], w2_d[:, :].bitcast(f32r))

            # ---- full-x load in chunks; DVE per-chunk min & max reduces
            # (Pool firmware has no free-axis reduce / tensor max)
            xf = big_pool.tile([128, FULL_COLS], f32)
            pq = tiny_pool.tile([128, 2 * N_CHUNKS], f32)  # [-min | max] per chunk
            for ci in range(N_CHUNKS):
                lo = ci * CHW
                hi = FULL_COLS if ci == N_CHUNKS - 1 else lo + CHW
                nc.sync.dma_start(xf[:, lo:hi], x_full[:, lo:hi])
                nc.vector.tensor_reduce(
                    pq[:, ci:ci + 1], xf[:, lo:hi], mybir.AxisListType.X,
                    mybir.AluOpType.min, negate=True,
                )
                nc.vector.tensor_reduce(
                    pq[:, N_CHUNKS + ci:N_CHUNKS + ci + 1], xf[:, lo:hi],
                    mybir.AxisListType.X, mybir.AluOpType.max,
                )

            # ---- PE clock ramp: dummy matmuls during phase A keep the PE
            # pipeline busy so real matmuls run at the full 2.4 GHz pstate
            dp = gpsum_pool.tile([2 * NB, 2 * BANK], f32, tag="gp")
            dmm_src = const_pool.tile([6, W], f32)
            nc.vector.memset(dmm_src[:, :], 1.0)
            for d in range(12):
                nc.tensor.matmul(
                    dp[:, (d % 2) * BANK:(d % 2) * BANK + W],
                    cu[0:6, :], dmm_src[:, :].bitcast(f32r),
                )

            # ---- min/max finalize
            pm = tiny_pool.tile([128, 2], f32)  # per-partition [-min, max]
            nc.vector.tensor_reduce(
                pm[:, 0:1], pq[:, 0:N_CHUNKS], mybir.AxisListType.X,
                mybir.AluOpType.max,
            )
            nc.vector.tensor_reduce(
                pm[:, 1:2], pq[:, N_CHUNKS:2 * N_CHUNKS], mybir.AxisListType.X,
                mybir.AluOpType.max,
            )
            g = tiny_pool.tile([1, 4], f32)  # [center, inv, max, span]
            nc.gpsimd.tensor_reduce(
                g[0:1, 0:1], pm[:, 0:1], mybir.AxisListType.XYZWC,
                mybir.AluOpType.max,
            )
            nc.gpsimd.tensor_reduce(
                g[0:1, 2:3], pm[:, 1:2], mybir.AxisListType.XYZWC,
                mybir.AluOpType.max,
            )
            # g0 currently holds -min: span = (max + (-min)) + 1e-8
            nc.vector.tensor_scalar(
                g[0:1, 3:4], g[0:1, 2:3], g[0:1, 0:1], 1e-8,
                mybir.AluOpType.add, mybir.AluOpType.add,
            )
            nc.vector.reciprocal(g[0:1, 1:2], g[0:1, 3:4])
            # g0 := center = 0.5*span - (-min)  (powers are of y = xn - 0.5)
            nc.vector.scalar_tensor_tensor(
                g[0:1, 0:1], g[0:1, 3:4], 0.5, g[0:1, 0:1],
                mybir.AluOpType.mult, mybir.AluOpType.subtract,
            )
            # broadcast (center, inv) to all partitions via a K=1 matmul
            ones = tiny_pool.tile([1, 128], f32)
            nc.vector.memset(ones[:, :], 1.0)
            muinv_p = gpsum_pool.tile([128, 2], f32, tag="gp")
            nc.tensor.matmul(muinv_p[:, :], ones[:, :], g[0:1, 0:2])
            muinv = tiny_pool.tile([128, 2], f32)
            nc.scalar.copy(muinv[:, :], muinv_p[:, :])

            # ---- power table xp [128, 3*W]: blocks [y | y^2 | y^3]
            # (shard = first 490 columns of xf)
            xp = big_pool.tile([128, 3 * W], f32)
            nc.vector.tensor_scalar(
                xp[:, 0:W], xf[:, 0:W], muinv[:, 0:1], muinv[:, 1:2],
                mybir.AluOpType.subtract, mybir.AluOpType.mult,
            )
            nc.vector.tensor_mul(xp[:, W:2 * W], xp[:, 0:W], xp[:, 0:W])
            nc.vector.tensor_mul(xp[:, 2 * W:3 * W], xp[:, W:2 * W], xp[:, 0:W])

            # ---- main pipeline: 64 pairs of 490-point tiles (block-diag),
            # two pairs per iteration sharing 2-bank PSUM tiles
            n_pairs = N_TILES // 2
            n_iters = n_pairs // 2          # 32
            for gi in range(0, n_iters, GROUP // 2):
                stage = stage_pool.tile([2 * NB, GROUP * W], bf16)
                for it in range(gi, min(gi + GROUP // 2, n_iters)):
                    pb = 2 * it
                    sl = (it - gi) * 2 * W

                    # xr loads via SWDGE (gpsimd) only: HWDGE-path dma_start
                    # costs ~650ns of issuing-engine SEQ each, which at 128
                    # loads would swamp the SP sequencer
                    xr = xrows_pool.tile([38, W], f32r)
                    for h in (0, 1):
                        t0 = 2 * (pb + h)
                        nc.gpsimd.dma_start(
                            xr[32 * h:32 * h + 6, :],
                            xp[t0:t0 + 2, :].bitcast(f32r).rearrange(
                                "p (q c) -> p q c", q=3),
                        )

                    gp = gpsum_pool.tile([2 * NB, 2 * BANK], f32)
                    nc.tensor.matmul(gp[:, 0:W], cu[0:6, :], xr[0:6, :])
                    nc.tensor.matmul(
                        gp[:, BANK:BANK + W], cu[32:38, :], xr[32:38, :]
                    )

                    stk = stack_pool.tile([2 * NB, 2 * W], f32r)
                    # clamp = Relu(in + bias) on every row; alternate engines
                    # so DVE and ACT each carry half of the two evacuation
                    # passes (clamp + evict) per iteration
                    stk_ap = stk[:, :].rearrange("r (p c) -> r p c", c=W)
                    gp_ap = gp[:, :].rearrange(
                        "r (p c) -> r p c", c=BANK)[:, :, 0:W]
                    if it % 2 == 0:
                        nc.vector.tensor_scalar(
                            stk_ap, gp_ap, bias[:, :], 0.0,
                            mybir.AluOpType.add, mybir.AluOpType.max,
                        )
                    else:
                        nc.scalar.activation(
                            stk_ap, gp_ap, mybir.ActivationFunctionType.Relu,
                            bias=bias[:, :],
                        )

                    op = opsum_pool.tile([2 * NB, 2 * BANK], f32)
                    nc.tensor.matmul(op[:, 0:W], w2t[:, :], stk[:, 0:W])
                    nc.tensor.matmul(
                        op[:, BANK:BANK + W], w2t[:, :], stk[:, W:2 * W]
                    )

                    # evict (+ evb bias) PSUM -> bf16 staging, opposite engine
                    st_ap = stage[:, sl:sl + 2 * W].rearrange(
                        "r (p c) -> r p c", c=W)
                    op_ap = op[:, :].rearrange(
                        "r (p c) -> r p c", c=BANK)[:, :, 0:W]
                    if it % 2 == 0:
                        nc.scalar.activation(
                            st_ap, op_ap, mybir.ActivationFunctionType.Identity,
                            bias=evb[:, :],
                        )
                    else:
                        nc.vector.tensor_scalar(
                            st_ap, op_ap, evb[:, :], None,
                            mybir.AluOpType.add,
                        )

                # pair-major blocks; host un-interleaves columns
                gw = 2 * min(GROUP // 2, n_iters - gi)   # pairs in group
                c0 = 2 * gi * 2 * W
                nc.sync.dma_start(
                    out_t[:, c0:c0 + gw * W], stage[0:NB, 0:gw * W]
                )
                nc.sync.dma_start(
                    out_t[:, c0 + gw * W:c0 + 2 * gw * W],
                    stage[NB:2 * NB, 0:gw * W],
                )

    nc.compile()
    return nc


# ------------------------------------------------------------------- driver
def _run(in_maps, trace=False):
    from concourse.bass_utils import run_bass_kernel_spmd

    if "nc" not in _CACHE:
        _CACHE["nc"] = _build_nc()
    return run_bass_kernel_spmd(
        _CACHE["nc"], in_maps, list(range(N_CORES)), trace=trace
    )


def _default_knots():
    inner = np.linspace(0.0, 1.0, NUM_KNOTS - 2 * DEGREE)
    return np.concatenate(
        [np.zeros(DEGREE), inner, np.ones(DEGREE)]
    ).astype(np.float32)


def kernel(x, knots=None, degree=None, _trace=False, _return_results=False, **_):
    x = np.asarray(x, np.float32).reshape(-1)
    assert x.size == N_POINTS
    if knots is None:
        knots = _default_knots()
    cube3x, bias124, evb, w2, row1 = _make_const_arrays(
        np.asarray(knots, np.float64))

    in_maps = []
    rest_cols = FULL_COLS - TILE_W
    for c in range(N_CORES):
        # full x per core: columns 0:490 hold the core's shard (tile-major,
        # partition p = tile p), remaining columns hold the rest of x
        xf2 = np.empty((128, FULL_COLS), np.float32)
        sh = np.empty(SHARD_PAD, np.float32)
        sh[:SHARD] = x[c * SHARD:(c + 1) * SHARD]
        sh[SHARD:] = x[c * SHARD]
        xf2[:, :TILE_W] = sh.reshape(128, TILE_W)
        rest = np.empty(128 * rest_cols, np.float32)
        nrest = N_POINTS - SHARD
        rest[:c * SHARD] = x[:c * SHARD]
        rest[c * SHARD:nrest] = x[(c + 1) * SHARD:]
        rest[nrest:] = x[0]
        xf2[:, TILE_W:] = rest.reshape(128, rest_cols)
        in_maps.append({
            "x_full": xf2,
            "cube3x": cube3x,
            "bias124": bias124,
            "evb": evb,
            "w2": w2,
        })

    res = _run(in_maps, trace=_trace)
    # device column -> local point index (pair-major group blocks)
    W = TILE_W
    n_pairs = N_TILES // 2
    perm = np.empty(SHARD_PAD, np.int64)
    col = 0
    for g in range(0, n_pairs, GROUP):
        gw = min(GROUP, n_pairs - g)
        for half in (0, 1):
            for pp in range(gw):
                t = 2 * (g + pp) + half
                perm[col:col + W] = t * W + np.arange(W)
                col += W
    out = np.empty((N_POINTS, NB), np.float32)
    full = np.empty((SHARD_PAD, NB), np.float32)
    for c in range(N_CORES):
        full[perm, :] = np.asarray(res.results[c]["out_t"]).astype(np.float32).T
        out[c * SHARD:(c + 1) * SHARD, :] = full[:SHARD]

    # boundary fixup: at xn == 1.0 exactly the reference jumps to the
    # degenerate right-end pieces (s = 64); patch those rows exactly
    mn, mx = x.min(), x.max()
    xn = (x - mn) / ((mx - mn) + np.float32(1e-8))
    at_one = np.nonzero(xn == np.float32(1.0))[0]
    if at_one.size:
        out[at_one, :] = row1.astype(np.float32)[None, :]

    if _return_results:
        return out, res
    return out
